# revision 5
# baseline (speedup 1.0000x reference)
"""ForgetMult linear recurrence h_t = f_t*x_t + (1-f_t)*h_{t-1} on 8 trn2 cores.

Sharding: batch dim B=64 split across 8 cores (8 batches/core). Per core the
C = 8*1024 = 8192 (b,h) channels are independent scans over T=1024.

All wire traffic fp16 (host pre-transposes to channel-major [C, T]; layout/
dtype prep only, all math on device). Engine assignment, from measured rates:
  - DVE owns the 64 tensor_tensor_scan ops (scan only runs on DVE; fp32
    operands measure fastest at ~3.79us/group) -> ~243us, the wall.
  - GpSimd does b = f*x (fp16 -> fp32 out), on 2-group [128, 2048] tiles to
    amortize per-instr overhead.
  - ACT does a = 1 - f (fp16 -> fp32 out) on 2-group tiles, plus output DMA.
"""

import numpy as np

import concourse.bacc as bacc
import concourse.bass as bass
import concourse.mybir as mybir
from concourse import bass_utils
from concourse.tile import TileContext

T = 1024
B = 64
H = 1024
NCORES = 8
BS = B // NCORES  # batches per core
C = BS * H  # channels per core (independent scans)
G = 128  # channels per group == partition dim
NG = C // G  # channel groups per core
PAIR = 2 * T  # free width of a 2-group tile

F32 = mybir.dt.float32
F16 = mybir.dt.float16


def build_program() -> bass.Bass:
    nc = bacc.Bacc(trn_type="TRN2")
    f_d = nc.dram_tensor("f", (C, T), F16, kind="ExternalInput")
    x_d = nc.dram_tensor("x", (C, T), F16, kind="ExternalInput")
    h0_d = nc.dram_tensor("h0", (G, NG), F32, kind="ExternalInput")
    y_d = nc.dram_tensor("y", (C, T), F16, kind="ExternalOutput")

    ACT = mybir.ActivationFunctionType.Copy
    MULT = mybir.AluOpType.mult
    ADD = mybir.AluOpType.add

    with TileContext(nc) as tc:
        with (
            tc.tile_pool(name="consts", bufs=1) as consts,
            tc.tile_pool(name="io", bufs=6) as io,
            tc.tile_pool(name="mid", bufs=3) as mid,
            tc.tile_pool(name="hpool", bufs=3) as hpool,
        ):
            h0t = consts.tile([G, NG], F32)
            nc.sync.dma_start(out=h0t[:, :], in_=h0_d[:, :])

            for p in range(NG // 2):  # 2 groups per tile
                g0 = 2 * p
                rows = slice(g0 * G, (g0 + 2) * G)
                ft = io.tile([G, PAIR], F16, tag="f")
                xt = io.tile([G, PAIR], F16, tag="x")
                for i in range(2):
                    grows = slice((g0 + i) * G, (g0 + i + 1) * G)
                    cl = slice(i * T, (i + 1) * T)
                    nc.sync.dma_start(out=ft[:, cl], in_=f_d[grows, :])
                    nc.sync.dma_start(out=xt[:, cl], in_=x_d[grows, :])
                at = mid.tile([G, PAIR], F32, tag="a")
                bt = mid.tile([G, PAIR], F32, tag="b")
                nc.scalar.activation(
                    at[:, :], ft[:, :], ACT, bias=1.0, scale=-1.0
                )
                nc.gpsimd.tensor_tensor(
                    out=bt[:, :], in0=ft[:, :], in1=xt[:, :], op=MULT
                )
                ht = hpool.tile([G, PAIR], F16, tag="h")
                for i in range(2):
                    g = g0 + i
                    cl = slice(i * T, (i + 1) * T)
                    nc.vector.tensor_tensor_scan(
                        out=ht[:, cl],
                        data0=at[:, cl],
                        data1=bt[:, cl],
                        initial=h0t[:, g : g + 1],
                        op0=MULT,
                        op1=ADD,
                    )
                for i in range(2):
                    grows = slice((g0 + i) * G, (g0 + i + 1) * G)
                    cl = slice(i * T, (i + 1) * T)
                    nc.scalar.dma_start(out=y_d[grows, :], in_=ht[:, cl])
    if not nc.is_finalized():
        nc.finalize()
    return nc


def run(inputs: dict, trace: bool = False, tmpdir=None) -> tuple[np.ndarray, object]:
    f = np.asarray(inputs["f"], dtype=np.float32)
    x = np.asarray(inputs["x"], dtype=np.float32)
    h0 = np.asarray(inputs["hidden_init"], dtype=np.float32)

    ftr = f.astype(np.float16).transpose(1, 2, 0)  # (B, H, T)
    xtr = x.astype(np.float16).transpose(1, 2, 0)

    nc = build_program()
    in_maps = []
    for m in range(NCORES):
        sl = slice(m * BS, (m + 1) * BS)
        in_maps.append(
            {
                "f": np.ascontiguousarray(ftr[sl]).reshape(C, T),
                "x": np.ascontiguousarray(xtr[sl]).reshape(C, T),
                "h0": np.ascontiguousarray(h0[sl].reshape(NG, G).T),
            }
        )
    res = bass_utils.run_bass_kernel_spmd(
        nc, in_maps, core_ids=list(range(NCORES)), trace=trace, tmpdir=tmpdir
    )
    outs = [r["y"].reshape(BS, H, T).transpose(2, 0, 1) for r in res.results]
    return np.concatenate(outs, axis=1).astype(np.float32), res


def kernel(**inputs) -> np.ndarray:
    out, _ = run(inputs, trace=False)
    return out
